# revision 5
# baseline (speedup 1.0000x reference)
"""Trainium2 Bass kernel for the Dupire local-vol Monte Carlo path simulation.

Reference recurrence (per path, 255 sequential steps):
    y     = sqrt(S/S0 + XS) * (t_k + TS)
    sigma = SB + y*exp(-y)
    S'    = S + r*S*dt + sigma*S*dW_k

Sharding: pure data parallel over the M=262144 paths -> 8 cores x 32768 paths.
Per core the 32768 paths live in SBUF as a [128, 256] f32 tile.

Key engine facts driving the design (TRN2):
  - exp and sqrt live in DIFFERENT ACT table sets (switch costs ~2.7us), so
    sqrt is computed as exp(0.5*ln(u)) using the natural_log_exp_and_others
    set: per step ACT does Ln, Exp(0.5*L), Exp(-c*r) -- one table set, no
    reloads.
  - Paths are split into two column halves [128,128] so ACT works on one half
    while DVE works on the other (otherwise the per-step dependency chain
    serializes the engines).
  - dW loads and S stores are batched K=32 time steps per DMA (4 MiB).
"""

import numpy as np

import concourse.bass as bass
import concourse.bacc as bacc
import concourse.tile as tile
from concourse import mybir
from concourse.bass_utils import run_bass_kernel_spmd

# Problem constants (match reference.py)
M = 262144
N_T = 256
DT = 0.004
S0 = 100.0
R_RATE = 0.05
SIGMA_BASE = 0.3
X_SHIFT = 0.1
T_SHIFT = 0.1

N_CORES = 8
M_CORE = M // N_CORES          # 32768 paths per core
P = 128                        # SBUF partitions
COLS = M_CORE // P             # 256 path-columns per partition
HALF = COLS // 2               # 128: column split for ACT/DVE overlap

AF = mybir.ActivationFunctionType
ALU = mybir.AluOpType


def _time_grid(n_t):
    # t_all = jnp.linspace(0, N_t*dt, N_t) in f32, as in the reference
    return np.linspace(0.0, n_t * DT, n_t).astype(np.float32)


def build(n_t=N_T, chunk=32):
    """Build the SPMD Bass module. Each core sees dW [n_t, 128, 256] and
    produces S [n_t, 128, 256]."""
    assert n_t % chunk == 0
    n_chunks = n_t // chunk
    t_all = _time_grid(n_t)
    k_drift = float(np.float32(1.0) + np.float32(R_RATE) * np.float32(DT))

    nc = bacc.Bacc("TRN2", target_bir_lowering=False, debug=False,
                   num_devices=N_CORES)
    # Register a const AP for the Ln bias (activation converts float biases
    # to per-partition const APs; only 0.0/1.0 are pre-registered).
    _const = nc.alloc_sbuf_tensor(f"const-f32-{X_SHIFT}", [P, 1],
                                  mybir.dt.float32)
    nc.gpsimd.memset(_const.ap(), X_SHIFT)
    nc.const_aps.aps[(mybir.dt.float32, X_SHIFT)] = _const.ap()
    nc.all_engine_barrier()

    dW_ext = nc.dram_tensor("dW", [n_t, P, COLS], mybir.dt.float32,
                            kind="ExternalInput")
    S_ext = nc.dram_tensor("S", [n_t, P, COLS], mybir.dt.float32,
                           kind="ExternalOutput")

    with tile.TileContext(nc) as tc:
        with tc.tile_pool(name="dw", bufs=2) as dw_pool, \
             tc.tile_pool(name="out", bufs=2) as o_pool, \
             tc.tile_pool(name="tmp", bufs=4) as tmp_pool:

            dw_prev = None
            prev = None  # AP of S_{r-1} tile [128, COLS]
            for c in range(n_chunks):
                dw_t = dw_pool.tile([P, chunk, COLS], mybir.dt.float32, tag="dw")
                nc.sync.dma_start(
                    out=dw_t[:],
                    in_=dW_ext[c * chunk:(c + 1) * chunk].rearrange("k p n -> p k n"),
                )
                o_t = o_pool.tile([P, chunk, COLS], mybir.dt.float32, tag="o")

                if c == 0:
                    nc.vector.memset(o_t[:, 0, :], S0)
                    prev = o_t[:, 0, :]
                    krange = range(1, chunk)
                else:
                    krange = range(0, chunk)

                for k in krange:
                    step = c * chunk + k - 1      # time index of this update
                    if k == 0:
                        dw_slice = dw_prev[:, chunk - 1, :]
                    else:
                        dw_slice = dw_t[:, k - 1, :]
                    c_t = float(np.float32(t_all[step]) + np.float32(T_SHIFT))

                    for h in range(2):
                        cs = slice(HALF * h, HALF * (h + 1))
                        s_prev = prev[:, cs]
                        L = tmp_pool.tile([P, HALF], mybir.dt.float32, tag=f"L{h}")
                        # L = ln(S/S0 + XS)
                        nc.scalar.activation(L[:], s_prev, AF.Ln,
                                             bias=X_SHIFT, scale=1.0 / S0)
                        Rt = tmp_pool.tile([P, HALF], mybir.dt.float32, tag=f"R{h}")
                        # r = exp(0.5*L) = sqrt(u)
                        nc.scalar.activation(Rt[:], L[:], AF.Exp,
                                             bias=0.0, scale=0.5)
                        E = tmp_pool.tile([P, HALF], mybir.dt.float32, tag=f"E{h}")
                        # e = exp(-c_t * r) = exp(-y)
                        nc.scalar.activation(E[:], Rt[:], AF.Exp,
                                             bias=0.0, scale=-c_t)
                        Q = tmp_pool.tile([P, HALF], mybir.dt.float32, tag=f"Q{h}")
                        # q = (r*c_t)*e = y*exp(-y)
                        nc.vector.scalar_tensor_tensor(Q[:], Rt[:], c_t, E[:],
                                                       ALU.mult, ALU.mult)
                        G = tmp_pool.tile([P, HALF], mybir.dt.float32, tag=f"G{h}")
                        # g = (q + SB)*dW = sigma*dW
                        nc.vector.scalar_tensor_tensor(G[:], Q[:], SIGMA_BASE,
                                                       dw_slice[:, cs],
                                                       ALU.add, ALU.mult)
                        # S' = (g + (1+r*dt))*S
                        nc.vector.scalar_tensor_tensor(o_t[:, k, cs], G[:],
                                                       k_drift, s_prev,
                                                       ALU.add, ALU.mult)
                    prev = o_t[:, k, :]

                nc.sync.dma_start(
                    out=S_ext[c * chunk:(c + 1) * chunk].rearrange("k p n -> p k n"),
                    in_=o_t[:],
                )
                dw_prev = dw_t
    nc.compile()
    return nc


_CACHED = {}


def _get_nc(n_t=N_T, chunk=32):
    key = (n_t, chunk)
    if key not in _CACHED:
        _CACHED[key] = build(n_t, chunk)
    return _CACHED[key]


def _shard(dW):
    """Full dW [N_T, M] -> per-core [N_T, 128, 256] slabs."""
    dW = np.ascontiguousarray(np.asarray(dW, dtype=np.float32))
    n_t = dW.shape[0]
    slabs = []
    for c in range(N_CORES):
        slab = dW[:, c * M_CORE:(c + 1) * M_CORE].reshape(n_t, P, COLS)
        slabs.append(np.ascontiguousarray(slab))
    return slabs


def _unshard(results, n_t):
    outs = [np.asarray(r["S"]).reshape(n_t, M_CORE) for r in results]
    return np.concatenate(outs, axis=1)


def run(dW, trace=False, chunk=32):
    """Run the SPMD kernel on 8 cores. Returns (S_full, BassKernelResults)."""
    dW = np.asarray(dW, dtype=np.float32)
    n_t = dW.shape[0]
    nc = _get_nc(n_t, chunk)
    in_maps = [{"dW": slab} for slab in _shard(dW)]
    res = run_bass_kernel_spmd(nc, in_maps, core_ids=list(range(N_CORES)),
                               trace=trace)
    return _unshard(res.results, n_t), res


def kernel(dW):
    out, _ = run(dW, trace=False)
    return out


# revision 29
# speedup vs baseline: 6.8250x; 6.8250x over previous
"""Trainium2 Bass kernel for the Dupire local-vol Monte Carlo path simulation.

Reference recurrence (per path, 255 sequential steps):
    y     = sqrt(S/S0 + XS) * (t_k + TS)
    sigma = SB + y*exp(-y)
    S'    = S + r*S*dt + sigma*S*dW_k

Sharding: pure data parallel over the M=262144 paths -> 8 cores x 32768 paths.
Per core the 32768 paths live in SBUF as a [128, 256] f32 tile.

Key engine facts driving the design (TRN2):
  - exp and sqrt live in DIFFERENT ACT table sets (switch costs ~2.7us), so
    sqrt is computed as exp(0.5*ln(u)) using the natural_log_exp_and_others
    set: per step ACT does Ln, Exp(0.5*L), Exp(-c*r) -- one table set, no
    reloads.
  - Paths are split into two column halves [128,128] so ACT works on one half
    while DVE works on the other (otherwise the per-step dependency chain
    serializes the engines).
  - dW loads and S stores are batched K=32 time steps per DMA (4 MiB).
"""

import numpy as np

import concourse.bass as bass
import concourse.bacc as bacc
import concourse.tile as tile
from concourse import mybir
from concourse.bass_utils import run_bass_kernel_spmd
from concourse.tile_rust import add_dep_helper

# Problem constants (match reference.py)
M = 262144
N_T = 256
DT = 0.004
S0 = 100.0
R_RATE = 0.05
SIGMA_BASE = 0.3
X_SHIFT = 0.1
T_SHIFT = 0.1

N_CORES = 8
M_CORE = M // N_CORES          # 32768 paths per core
P = 128                        # SBUF partitions
COLS = M_CORE // P             # 256 path-columns per partition
HALF = COLS // 2               # 128: column split for ACT/DVE overlap

AF = mybir.ActivationFunctionType
ALU = mybir.AluOpType


def _time_grid(n_t):
    # t_all = jnp.linspace(0, N_t*dt, N_t) in f32, as in the reference
    return np.linspace(0.0, n_t * DT, n_t).astype(np.float32)


def build(n_t=N_T, chunk=32, reps=1, prio=True, chain=True, fast=True,
          dw_bufs=2, o_bufs=2, tmp_bufs=4, store_eng="sync"):
    """Build the SPMD Bass module. Each core sees dW [n_t, 128, 256] and
    produces S [n_t, 128, 256]. reps>1 wraps the whole computation in a
    hardware loop (identical output; used for wall-clock timing).

    prio=True assigns explicit scheduling priorities so each half's
    ACT trio (Ln,Exp,Exp) runs back-to-back and the two halves run
    half-period offset: ACT [L0 R0 E0][L1 R1 E1] while DVE runs the
    opposite half's [Q G S'] trio. Without this the Tile scheduler
    buckets the halves in phase (all L's, then R's, ... all S's),
    which serializes ACT bursts against DVE bursts (~2.9us/step instead
    of ~1.9us/step)."""
    assert n_t % chunk == 0
    n_chunks = n_t // chunk
    t_all = _time_grid(n_t)
    k_drift = float(np.float32(1.0) + np.float32(R_RATE) * np.float32(DT))

    nc = bacc.Bacc("TRN2", target_bir_lowering=False, debug=False,
                   num_devices=N_CORES)
    # Register a const AP for the Ln bias (activation converts float biases
    # to per-partition const APs; only 0.0/1.0 are pre-registered).
    _const = nc.alloc_sbuf_tensor(f"const-f32-{X_SHIFT}", [P, 1],
                                  mybir.dt.float32)
    nc.gpsimd.memset(_const.ap(), X_SHIFT)
    nc.const_aps.aps[(mybir.dt.float32, X_SHIFT)] = _const.ap()
    nc.all_engine_barrier()

    dW_ext = nc.dram_tensor("dW", [n_t, P, COLS], mybir.dt.float32,
                            kind="ExternalInput")
    S_ext = nc.dram_tensor("S", [n_t, P, COLS], mybir.dt.float32,
                           kind="ExternalOutput")

    from contextlib import ExitStack
    with tile.TileContext(nc) as tc, ExitStack() as stack:
        if reps > 1:
            stack.enter_context(tc.For_i(0, reps, 1))
        with tc.tile_pool(name="dw", bufs=dw_bufs) as dw_pool, \
             tc.tile_pool(name="out", bufs=o_bufs) as o_pool, \
             tc.tile_pool(name="tmp", bufs=tmp_bufs) as tmp_pool:

            dw_prev = None
            a_prev = None
            prev = None  # AP of S_{r-1} tile [128, COLS]
            for c in range(n_chunks):
                dw_t = dw_pool.tile([P, chunk, COLS], mybir.dt.float32, tag="dw")
                nc.sync.dma_start(
                    out=dw_t[:],
                    in_=dW_ext[c * chunk:(c + 1) * chunk].rearrange("k p n -> p k n"),
                )
                a_t = None
                if fast:
                    # A = 0.3*dW + (1+r*dt), batched over the whole chunk:
                    # the drift+base-vol part of the update, off the
                    # per-step critical path.
                    a_t = dw_pool.tile([P, chunk, COLS], mybir.dt.float32,
                                       tag="a")
                    nc.vector.tensor_scalar(
                        a_t[:].rearrange("p k n -> p (k n)"),
                        dw_t[:].rearrange("p k n -> p (k n)"),
                        SIGMA_BASE, k_drift, ALU.mult, ALU.add)
                o_t = o_pool.tile([P, chunk, COLS], mybir.dt.float32, tag="o")

                if c == 0:
                    nc.vector.memset(o_t[:, 0, :], S0)
                    prev = o_t[:, 0, :]
                    krange = range(1, chunk)
                else:
                    krange = range(0, chunk)

                for k in krange:
                    step = c * chunk + k - 1      # time index of this update
                    if k == 0:
                        dw_slice = dw_prev[:, chunk - 1, :]
                        a_slice = a_prev[:, chunk - 1, :] if fast else None
                    else:
                        dw_slice = dw_t[:, k - 1, :]
                        a_slice = a_t[:, k - 1, :] if fast else None
                    c_t = float(np.float32(t_all[step]) + np.float32(T_SHIFT))

                    base = 1_000_000 + (c * chunk + k) * 100
                    e_prev_half = None
                    for h in range(2):
                        cs = slice(HALF * h, HALF * (h + 1))
                        s_prev = prev[:, cs]
                        L = tmp_pool.tile([P, HALF], mybir.dt.float32, tag=f"L{h}")
                        # L = ln(S/S0 + XS)
                        i0 = nc.scalar.activation(L[:], s_prev, AF.Ln,
                                                  bias=X_SHIFT, scale=1.0 / S0)
                        Rt = tmp_pool.tile([P, HALF], mybir.dt.float32, tag=f"R{h}")
                        # r = exp(0.5*L) = sqrt(u)
                        i1 = nc.scalar.activation(Rt[:], L[:], AF.Exp,
                                                  bias=0.0, scale=0.5)
                        E = tmp_pool.tile([P, HALF], mybir.dt.float32, tag=f"E{h}")
                        # e = exp(-c_t * r) = exp(-y)
                        i2 = nc.scalar.activation(E[:], Rt[:], AF.Exp,
                                                  bias=0.0, scale=-c_t)
                        if fast:
                            # Critical path from e is only 2 DVE ops:
                            #   S' = (yc*dW*S)*e + (0.3*dW + k_drift)*S
                            # with W2=dW*S, AS=A*S computed at step start and
                            # yc=c*sqrt(u), B2S=yc*W2 right after the R op.
                            W2 = tmp_pool.tile([P, HALF], mybir.dt.float32,
                                               tag=f"W2{h}")
                            j0 = nc.vector.tensor_tensor(W2[:], dw_slice[:, cs],
                                                         s_prev, ALU.mult)
                            AS = tmp_pool.tile([P, HALF], mybir.dt.float32,
                                               tag=f"AS{h}")
                            j1 = nc.vector.tensor_tensor(AS[:], a_slice[:, cs],
                                                         s_prev, ALU.mult)
                            B2S = tmp_pool.tile([P, HALF], mybir.dt.float32,
                                                tag=f"B2S{h}")
                            # yc*W2 = (c*r)*dW*S, folded into one stt
                            j2 = nc.vector.scalar_tensor_tensor(
                                B2S[:], Rt[:], c_t, W2[:], ALU.mult, ALU.mult)
                            Hh = tmp_pool.tile([P, HALF], mybir.dt.float32,
                                               tag=f"H{h}")
                            i4 = nc.vector.tensor_tensor(Hh[:], B2S[:], E[:],
                                                         ALU.mult)
                            i5 = nc.vector.tensor_tensor(o_t[:, k, cs], Hh[:],
                                                         AS[:], ALU.add)
                            if prio:
                                j0.ins.bass_priority = base + 10 * h + 3
                                j1.ins.bass_priority = base + 10 * h + 4
                                j2.ins.bass_priority = base + 10 * h + 5
                        else:
                            Q = tmp_pool.tile([P, HALF], mybir.dt.float32, tag=f"Q{h}")
                            # q = (r*c_t)*e = y*exp(-y)
                            i3 = nc.vector.scalar_tensor_tensor(Q[:], Rt[:], c_t, E[:],
                                                                ALU.mult, ALU.mult)
                            G = tmp_pool.tile([P, HALF], mybir.dt.float32, tag=f"G{h}")
                            # g = (q + SB)*dW = sigma*dW
                            i4 = nc.vector.scalar_tensor_tensor(G[:], Q[:], SIGMA_BASE,
                                                                dw_slice[:, cs],
                                                                ALU.add, ALU.mult)
                            # S' = (g + (1+r*dt))*S
                            i5 = nc.vector.scalar_tensor_tensor(o_t[:, k, cs], G[:],
                                                                k_drift, s_prev,
                                                                ALU.add, ALU.mult)
                            if prio:
                                i3.ins.bass_priority = base + 20 + 10 * h + 0
                        if prio:
                            i0.ins.bass_priority = base + 10 * h + 0
                            i1.ins.bass_priority = base + 10 * h + 1
                            i2.ins.bass_priority = base + 10 * h + 2
                            i4.ins.bass_priority = base + 20 + 10 * h + 1
                            i5.ins.bass_priority = base + 20 + 10 * h + 2
                        if chain and e_prev_half is not None:
                            # Half-offset software pipeline: half-1's ACT trio
                            # starts only after half-0's E, so DVE(half-0)
                            # overlaps ACT(half-1). Scheduling-only edge
                            # (same engine, in-order at runtime).
                            add_dep_helper(i0.ins, e_prev_half.ins, sync=False,
                                           reason="half-offset pipeline")
                        e_prev_half = i2
                    prev = o_t[:, k, :]

                store = nc.sync if store_eng == "sync" else nc.scalar
                store.dma_start(
                    out=S_ext[c * chunk:(c + 1) * chunk].rearrange("k p n -> p k n"),
                    in_=o_t[:],
                )
                dw_prev = dw_t
                a_prev = a_t
    _compile_with_one_act_table(nc)
    return nc


def _compile_with_one_act_table(nc):
    """nc.compile() with the ACT table-set list restricted to
    natural_log_exp_and_others. The default greedy insertion pass pairs Ln
    with the natural_log set and Exp with exp_and_others, reloading tables
    twice per step (2x255x1283ns = 654us!). All our activations are Ln/Exp,
    which the combined set covers with a single load at kernel entry.
    Indices into act_info.json's act_func_sets are preserved (other entries
    are emptied, not removed)."""
    target = "natural_log_exp_and_others"
    orig = bacc.get_activation_tables

    def patched(arch):
        full = orig(arch)
        assert target in full, sorted(full)
        return {name: (fns if name == target else set())
                for name, fns in full.items()}

    bacc.get_activation_tables = patched
    try:
        nc.compile()
    finally:
        bacc.get_activation_tables = orig


_CACHED = {}


def _get_nc(n_t=N_T, chunk=16, reps=1, prio=True, chain=False, fast=True):
    key = (n_t, chunk, reps, prio, chain, fast)
    if key not in _CACHED:
        _CACHED[key] = build(n_t, chunk, reps, prio, chain, fast)
    return _CACHED[key]


def _shard(dW):
    """Full dW [N_T, M] -> per-core [N_T, 128, 256] slabs."""
    dW = np.ascontiguousarray(np.asarray(dW, dtype=np.float32))
    n_t = dW.shape[0]
    slabs = []
    for c in range(N_CORES):
        slab = dW[:, c * M_CORE:(c + 1) * M_CORE].reshape(n_t, P, COLS)
        slabs.append(np.ascontiguousarray(slab))
    return slabs


def _unshard(results, n_t):
    outs = [np.asarray(r["S"]).reshape(n_t, M_CORE) for r in results]
    return np.concatenate(outs, axis=1)


def run(dW, trace=False, chunk=16):
    """Run the SPMD kernel on 8 cores. Returns (S_full, BassKernelResults)."""
    dW = np.asarray(dW, dtype=np.float32)
    n_t = dW.shape[0]
    nc = _get_nc(n_t, chunk)
    in_maps = [{"dW": slab} for slab in _shard(dW)]
    res = run_bass_kernel_spmd(nc, in_maps, core_ids=list(range(N_CORES)),
                               trace=trace)
    return _unshard(res.results, n_t), res


def kernel(dW):
    out, _ = run(dW, trace=False)
    return out


# revision 34
# speedup vs baseline: 7.9934x; 1.1712x over previous
"""Trainium2 Bass kernel for the Dupire local-vol Monte Carlo path simulation.

Reference recurrence (per path, 255 sequential steps):
    y     = sqrt(S/S0 + XS) * (t_k + TS)
    sigma = SB + y*exp(-y)
    S'    = S + r*S*dt + sigma*S*dW_k

Sharding: pure data parallel over the M=262144 paths -> 8 cores x 32768 paths.
Per core the 32768 paths live in SBUF as a [128, 256] f32 tile.

Key engine facts driving the design (TRN2):
  - exp and sqrt live in DIFFERENT ACT table sets (switch costs ~2.7us; the
    default bacc insertion pass even reloads 2x per step = +654us), so sqrt
    is computed as exp(0.5*ln(u)) using the natural_log_exp_and_others set
    (forced via _compile_with_one_act_table): one table load total.
  - Paths are split into two column halves [128,128] so ACT works on one half
    while DVE works on the other (otherwise the per-step dependency chain
    serializes the engines). The kernel is latency-bound on the cross-engine
    cycle E->H->S'->L (ACT ~67% busy), not throughput-bound.
  - The DVE critical tail from e=exp(-y) is only 2 ops (fast=True):
        S' = (c*r*dW*S)*e + (0.3*dW + 1+r*dt)*S
    with dW*S, A*S, (c*r)*(dW*S) precomputed off the critical path and
    A = 0.3*dW + (1+r*dt) batched per chunk.
  - dW loads and S stores are batched K=16 time steps per DMA (2 MiB),
    double-buffered; DMA (~186us busy) hides fully under compute.
  - Explicit bass_priority hints give the Tile scheduler the intended
    software-pipeline order (~5% better than without).

Measured on 8 axon trn2 cores: ~660-800 us per full kernel (session noise),
cost-model prediction 670 us; ACT-busy floor is 448 us, HBM roofline 187 us.
"""

import numpy as np

import concourse.bass as bass
import concourse.bacc as bacc
import concourse.tile as tile
from concourse import mybir
from concourse.bass_utils import run_bass_kernel_spmd
from concourse.tile_rust import add_dep_helper

# Problem constants (match reference.py)
M = 262144
N_T = 256
DT = 0.004
S0 = 100.0
R_RATE = 0.05
SIGMA_BASE = 0.3
X_SHIFT = 0.1
T_SHIFT = 0.1

N_CORES = 8
M_CORE = M // N_CORES          # 32768 paths per core
P = 128                        # SBUF partitions
COLS = M_CORE // P             # 256 path-columns per partition
HALF = COLS // 2               # 128: column split for ACT/DVE overlap

AF = mybir.ActivationFunctionType
ALU = mybir.AluOpType


def _time_grid(n_t):
    # t_all = jnp.linspace(0, N_t*dt, N_t) in f32, as in the reference
    return np.linspace(0.0, n_t * DT, n_t).astype(np.float32)


def build(n_t=N_T, chunk=32, reps=1, prio=True, chain=True, fast=True,
          dw_bufs=2, o_bufs=2, tmp_bufs=4, store_eng="sync", wide=False,
          period=None, t0=30000):
    """Build the SPMD Bass module. Each core sees dW [n_t, 128, 256] and
    produces S [n_t, 128, 256]. reps>1 wraps the whole computation in a
    hardware loop (identical output; used for wall-clock timing).

    prio=True assigns explicit scheduling priorities so each half's
    ACT trio (Ln,Exp,Exp) runs back-to-back and the two halves run
    half-period offset: ACT [L0 R0 E0][L1 R1 E1] while DVE runs the
    opposite half's [Q G S'] trio. Without this the Tile scheduler
    buckets the halves in phase (all L's, then R's, ... all S's),
    which serializes ACT bursts against DVE bursts (~2.9us/step instead
    of ~1.9us/step)."""
    assert n_t % chunk == 0
    n_chunks = n_t // chunk
    t_all = _time_grid(n_t)
    k_drift = float(np.float32(1.0) + np.float32(R_RATE) * np.float32(DT))

    nc = bacc.Bacc("TRN2", target_bir_lowering=False, debug=False,
                   num_devices=N_CORES)
    # Register a const AP for the Ln bias (activation converts float biases
    # to per-partition const APs; only 0.0/1.0 are pre-registered).
    _const = nc.alloc_sbuf_tensor(f"const-f32-{X_SHIFT}", [P, 1],
                                  mybir.dt.float32)
    nc.gpsimd.memset(_const.ap(), X_SHIFT)
    nc.const_aps.aps[(mybir.dt.float32, X_SHIFT)] = _const.ap()
    nc.all_engine_barrier()

    dW_ext = nc.dram_tensor("dW", [n_t, P, COLS], mybir.dt.float32,
                            kind="ExternalInput")
    S_ext = nc.dram_tensor("S", [n_t, P, COLS], mybir.dt.float32,
                           kind="ExternalOutput")

    from contextlib import ExitStack
    with tile.TileContext(nc) as tc, ExitStack() as stack:
        if reps > 1:
            stack.enter_context(tc.For_i(0, reps, 1))
        with tc.tile_pool(name="dw", bufs=dw_bufs) as dw_pool, \
             tc.tile_pool(name="out", bufs=o_bufs) as o_pool, \
             tc.tile_pool(name="tmp", bufs=tmp_bufs) as tmp_pool:

            dw_prev = None
            a_prev = None
            prev = None  # AP of S_{r-1} tile [128, COLS]
            for c in range(n_chunks):
                dw_t = dw_pool.tile([P, chunk, COLS], mybir.dt.float32, tag="dw")
                nc.sync.dma_start(
                    out=dw_t[:],
                    in_=dW_ext[c * chunk:(c + 1) * chunk].rearrange("k p n -> p k n"),
                )
                a_t = None
                if fast:
                    # A = 0.3*dW + (1+r*dt), batched over the whole chunk:
                    # the drift+base-vol part of the update, off the
                    # per-step critical path.
                    a_t = dw_pool.tile([P, chunk, COLS], mybir.dt.float32,
                                       tag="a")
                    nc.vector.tensor_scalar(
                        a_t[:].rearrange("p k n -> p (k n)"),
                        dw_t[:].rearrange("p k n -> p (k n)"),
                        SIGMA_BASE, k_drift, ALU.mult, ALU.add)
                o_t = o_pool.tile([P, chunk, COLS], mybir.dt.float32, tag="o")

                if c == 0:
                    nc.vector.memset(o_t[:, 0, :], S0)
                    prev = o_t[:, 0, :]
                    krange = range(1, chunk)
                else:
                    krange = range(0, chunk)

                for k in krange:
                    step = c * chunk + k - 1      # time index of this update
                    if k == 0:
                        dw_slice = dw_prev[:, chunk - 1, :]
                        a_slice = a_prev[:, chunk - 1, :] if fast else None
                    else:
                        dw_slice = dw_t[:, k - 1, :]
                        a_slice = a_t[:, k - 1, :] if fast else None
                    c_t = float(np.float32(t_all[step]) + np.float32(T_SHIFT))

                    base = 1_000_000 + (c * chunk + k) * 100
                    if wide:
                        # Fewer, larger instructions: per-half Ln (so each
                        # half's chain closes independently), one wide
                        # Exp(0.5L)=sqrt(u), per-half Exp(-c*r); wide DVE
                        # precompute, per-half 2-op critical tail.
                        Lw = tmp_pool.tile([P, COLS], mybir.dt.float32, tag="Lw")
                        iL0 = nc.scalar.activation(Lw[:, 0:HALF], prev[:, 0:HALF],
                                                   AF.Ln, bias=X_SHIFT,
                                                   scale=1.0 / S0)
                        iL1 = nc.scalar.activation(Lw[:, HALF:COLS],
                                                   prev[:, HALF:COLS],
                                                   AF.Ln, bias=X_SHIFT,
                                                   scale=1.0 / S0)
                        Rw = tmp_pool.tile([P, COLS], mybir.dt.float32, tag="Rw")
                        iR = nc.scalar.activation(Rw[:], Lw[:], AF.Exp,
                                                  bias=0.0, scale=0.5)
                        W2w = tmp_pool.tile([P, COLS], mybir.dt.float32, tag="W2w")
                        jW = nc.vector.tensor_tensor(W2w[:], dw_slice[:], prev,
                                                     ALU.mult)
                        ASw = tmp_pool.tile([P, COLS], mybir.dt.float32, tag="ASw")
                        jA = nc.vector.tensor_tensor(ASw[:], a_slice[:], prev,
                                                     ALU.mult)
                        B2Sw = tmp_pool.tile([P, COLS], mybir.dt.float32, tag="B2Sw")
                        jB = nc.vector.scalar_tensor_tensor(B2Sw[:], Rw[:], c_t,
                                                            W2w[:], ALU.mult,
                                                            ALU.mult)
                        if prio:
                            iL0.ins.bass_priority = base + 0
                            iL1.ins.bass_priority = base + 1
                            iR.ins.bass_priority = base + 2
                            jW.ins.bass_priority = base + 3
                            jA.ins.bass_priority = base + 4
                            jB.ins.bass_priority = base + 5
                        for h in range(2):
                            cs = slice(HALF * h, HALF * (h + 1))
                            E = tmp_pool.tile([P, HALF], mybir.dt.float32,
                                              tag=f"E{h}")
                            iE = nc.scalar.activation(E[:], Rw[:, cs], AF.Exp,
                                                      bias=0.0, scale=-c_t)
                            Hh = tmp_pool.tile([P, HALF], mybir.dt.float32,
                                               tag=f"H{h}")
                            iH = nc.vector.tensor_tensor(Hh[:], B2Sw[:, cs],
                                                         E[:], ALU.mult)
                            iS = nc.vector.tensor_tensor(o_t[:, k, cs], Hh[:],
                                                         ASw[:, cs], ALU.add)
                            if prio:
                                iE.ins.bass_priority = base + 10 + h
                                iH.ins.bass_priority = base + 20 + 2 * h
                                iS.ins.bass_priority = base + 21 + 2 * h
                        prev = o_t[:, k, :]
                        continue
                    e_prev_half = None
                    for h in range(2):
                        cs = slice(HALF * h, HALF * (h + 1))
                        s_prev = prev[:, cs]
                        L = tmp_pool.tile([P, HALF], mybir.dt.float32, tag=f"L{h}")
                        # L = ln(S/S0 + XS)
                        i0 = nc.scalar.activation(L[:], s_prev, AF.Ln,
                                                  bias=X_SHIFT, scale=1.0 / S0)
                        Rt = tmp_pool.tile([P, HALF], mybir.dt.float32, tag=f"R{h}")
                        # r = exp(0.5*L) = sqrt(u)
                        i1 = nc.scalar.activation(Rt[:], L[:], AF.Exp,
                                                  bias=0.0, scale=0.5)
                        E = tmp_pool.tile([P, HALF], mybir.dt.float32, tag=f"E{h}")
                        # e = exp(-c_t * r) = exp(-y)
                        i2 = nc.scalar.activation(E[:], Rt[:], AF.Exp,
                                                  bias=0.0, scale=-c_t)
                        if fast:
                            # Critical path from e is only 2 DVE ops:
                            #   S' = (yc*dW*S)*e + (0.3*dW + k_drift)*S
                            # with W2=dW*S, AS=A*S computed at step start and
                            # yc=c*sqrt(u), B2S=yc*W2 right after the R op.
                            W2 = tmp_pool.tile([P, HALF], mybir.dt.float32,
                                               tag=f"W2{h}")
                            j0 = nc.vector.tensor_tensor(W2[:], dw_slice[:, cs],
                                                         s_prev, ALU.mult)
                            AS = tmp_pool.tile([P, HALF], mybir.dt.float32,
                                               tag=f"AS{h}")
                            j1 = nc.vector.tensor_tensor(AS[:], a_slice[:, cs],
                                                         s_prev, ALU.mult)
                            B2S = tmp_pool.tile([P, HALF], mybir.dt.float32,
                                                tag=f"B2S{h}")
                            # yc*W2 = (c*r)*dW*S, folded into one stt
                            j2 = nc.vector.scalar_tensor_tensor(
                                B2S[:], Rt[:], c_t, W2[:], ALU.mult, ALU.mult)
                            Hh = tmp_pool.tile([P, HALF], mybir.dt.float32,
                                               tag=f"H{h}")
                            i4 = nc.vector.tensor_tensor(Hh[:], B2S[:], E[:],
                                                         ALU.mult)
                            i5 = nc.vector.tensor_tensor(o_t[:, k, cs], Hh[:],
                                                         AS[:], ALU.add)
                            if prio:
                                j0.ins.bass_priority = base + 10 * h + 3
                                j1.ins.bass_priority = base + 10 * h + 4
                                j2.ins.bass_priority = base + 10 * h + 5
                        else:
                            Q = tmp_pool.tile([P, HALF], mybir.dt.float32, tag=f"Q{h}")
                            # q = (r*c_t)*e = y*exp(-y)
                            i3 = nc.vector.scalar_tensor_tensor(Q[:], Rt[:], c_t, E[:],
                                                                ALU.mult, ALU.mult)
                            G = tmp_pool.tile([P, HALF], mybir.dt.float32, tag=f"G{h}")
                            # g = (q + SB)*dW = sigma*dW
                            i4 = nc.vector.scalar_tensor_tensor(G[:], Q[:], SIGMA_BASE,
                                                                dw_slice[:, cs],
                                                                ALU.add, ALU.mult)
                            # S' = (g + (1+r*dt))*S
                            i5 = nc.vector.scalar_tensor_tensor(o_t[:, k, cs], G[:],
                                                                k_drift, s_prev,
                                                                ALU.add, ALU.mult)
                            if prio:
                                i3.ins.bass_priority = base + 20 + 10 * h + 0
                        if prio:
                            i0.ins.bass_priority = base + 10 * h + 0
                            i1.ins.bass_priority = base + 10 * h + 1
                            i2.ins.bass_priority = base + 10 * h + 2
                            i4.ins.bass_priority = base + 20 + 10 * h + 1
                            i5.ins.bass_priority = base + 20 + 10 * h + 2
                        if period is not None and fast:
                            # manual schedule floors (scheduling hints only):
                            # bucketed ACT [L0 L1 R0 R1 E0 E1], DVE critical
                            # tail [H0 H1 S0' S1'] at the end of the period.
                            sb = t0 + (c * chunk + k) * period
                            i0.ins.bass_wait_until_ts = sb + 292 * h
                            i1.ins.bass_wait_until_ts = sb + 584 + 292 * h
                            i2.ins.bass_wait_until_ts = sb + 1168 + 292 * h
                            i4.ins.bass_wait_until_ts = sb + 1745 + 194 * h
                            i5.ins.bass_wait_until_ts = sb + 2133 + 194 * h
                        if chain and e_prev_half is not None:
                            # Half-offset software pipeline: half-1's ACT trio
                            # starts only after half-0's E, so DVE(half-0)
                            # overlaps ACT(half-1). Scheduling-only edge
                            # (same engine, in-order at runtime).
                            add_dep_helper(i0.ins, e_prev_half.ins, sync=False,
                                           reason="half-offset pipeline")
                        e_prev_half = i2
                    prev = o_t[:, k, :]

                store = nc.sync if store_eng == "sync" else nc.scalar
                store.dma_start(
                    out=S_ext[c * chunk:(c + 1) * chunk].rearrange("k p n -> p k n"),
                    in_=o_t[:],
                )
                dw_prev = dw_t
                a_prev = a_t
    _compile_with_one_act_table(nc)
    return nc


def _compile_with_one_act_table(nc):
    """nc.compile() with the ACT table-set list restricted to
    natural_log_exp_and_others. The default greedy insertion pass pairs Ln
    with the natural_log set and Exp with exp_and_others, reloading tables
    twice per step (2x255x1283ns = 654us!). All our activations are Ln/Exp,
    which the combined set covers with a single load at kernel entry.
    Indices into act_info.json's act_func_sets are preserved (other entries
    are emptied, not removed)."""
    target = "natural_log_exp_and_others"
    orig = bacc.get_activation_tables

    def patched(arch):
        full = orig(arch)
        assert target in full, sorted(full)
        return {name: (fns if name == target else set())
                for name, fns in full.items()}

    bacc.get_activation_tables = patched
    try:
        nc.compile()
    finally:
        bacc.get_activation_tables = orig


_CACHED = {}


def _get_nc(n_t=N_T, chunk=16, reps=1, prio=True, chain=False, fast=True):
    key = (n_t, chunk, reps, prio, chain, fast)
    if key not in _CACHED:
        _CACHED[key] = build(n_t, chunk, reps, prio, chain, fast)
    return _CACHED[key]


def _shard(dW):
    """Full dW [N_T, M] -> per-core [N_T, 128, 256] slabs."""
    dW = np.ascontiguousarray(np.asarray(dW, dtype=np.float32))
    n_t = dW.shape[0]
    slabs = []
    for c in range(N_CORES):
        slab = dW[:, c * M_CORE:(c + 1) * M_CORE].reshape(n_t, P, COLS)
        slabs.append(np.ascontiguousarray(slab))
    return slabs


def _unshard(results, n_t):
    outs = [np.asarray(r["S"]).reshape(n_t, M_CORE) for r in results]
    return np.concatenate(outs, axis=1)


def run(dW, trace=False, chunk=16):
    """Run the SPMD kernel on 8 cores. Returns (S_full, BassKernelResults)."""
    dW = np.asarray(dW, dtype=np.float32)
    n_t = dW.shape[0]
    nc = _get_nc(n_t, chunk)
    in_maps = [{"dW": slab} for slab in _shard(dW)]
    res = run_bass_kernel_spmd(nc, in_maps, core_ids=list(range(N_CORES)),
                               trace=trace)
    return _unshard(res.results, n_t), res


def kernel(dW):
    out, _ = run(dW, trace=False)
    return out


# revision 42
# speedup vs baseline: 8.9108x; 1.1148x over previous
"""Trainium2 Bass kernel for the Dupire local-vol Monte Carlo path simulation.

Reference recurrence (per path, 255 sequential steps):
    y     = sqrt(S/S0 + XS) * (t_k + TS)
    sigma = SB + y*exp(-y)
    S'    = S + r*S*dt + sigma*S*dW_k

Sharding: pure data parallel over the M=262144 paths -> 8 cores x 32768 paths.
Per core the 32768 paths live in SBUF as a [128, 256] f32 tile.

Key engine facts driving the design (TRN2):
  - exp and sqrt live in DIFFERENT ACT table sets (switch costs ~2.7us; the
    default bacc insertion pass even reloads 2x per step = +654us), so sqrt
    is computed as exp(0.5*ln(u)) using the natural_log_exp_and_others set
    (forced via _compile_with_one_act_table): one table load total.
  - Paths are split into two column halves [128,128] so ACT works on one half
    while DVE works on the other (otherwise the per-step dependency chain
    serializes the engines). The kernel is latency-bound on the cross-engine
    cycle E->H->S'->L (ACT ~67% busy), not throughput-bound.
  - The DVE critical tail from e=exp(-y) is only 2 ops (fast=True):
        S' = (c*r*dW*S)*e + (0.3*dW + 1+r*dt)*S
    with dW*S, A*S, (c*r)*(dW*S) precomputed off the critical path and
    A = 0.3*dW + (1+r*dt) batched per chunk.
  - dW loads and S stores are batched K=16 time steps per DMA (2 MiB),
    double-buffered; DMA (~186us busy) hides fully under compute.
  - Explicit bass_priority hints give the Tile scheduler the intended
    software-pipeline order (~5% better than without).

Measured on 8 axon trn2 cores: ~660-800 us per full kernel (session noise),
cost-model prediction 670 us; ACT-busy floor is 448 us, HBM roofline 187 us.
"""

import numpy as np

import concourse.bass as bass
import concourse.bacc as bacc
import concourse.tile as tile
from concourse import mybir
from concourse.bass_utils import run_bass_kernel_spmd
from concourse.tile_rust import add_dep_helper

# Problem constants (match reference.py)
M = 262144
N_T = 256
DT = 0.004
S0 = 100.0
R_RATE = 0.05
SIGMA_BASE = 0.3
X_SHIFT = 0.1
T_SHIFT = 0.1

N_CORES = 8
M_CORE = M // N_CORES          # 32768 paths per core
P = 128                        # SBUF partitions
COLS = M_CORE // P             # 256 path-columns per partition
HALF = COLS // 2               # 128: column split for ACT/DVE overlap

AF = mybir.ActivationFunctionType
ALU = mybir.AluOpType


def _time_grid(n_t):
    # t_all = jnp.linspace(0, N_t*dt, N_t) in f32, as in the reference
    return np.linspace(0.0, n_t * DT, n_t).astype(np.float32)


def build(n_t=N_T, chunk=32, reps=1, prio=True, chain=True, fast=True,
          dw_bufs=2, o_bufs=2, tmp_bufs=4, store_eng="sync", wide=False,
          period=None, t0=30000, psum=False):
    """Build the SPMD Bass module. Each core sees dW [n_t, 128, 256] and
    produces S [n_t, 128, 256]. reps>1 wraps the whole computation in a
    hardware loop (identical output; used for wall-clock timing).

    prio=True assigns explicit scheduling priorities so each half's
    ACT trio (Ln,Exp,Exp) runs back-to-back and the two halves run
    half-period offset: ACT [L0 R0 E0][L1 R1 E1] while DVE runs the
    opposite half's [Q G S'] trio. Without this the Tile scheduler
    buckets the halves in phase (all L's, then R's, ... all S's),
    which serializes ACT bursts against DVE bursts (~2.9us/step instead
    of ~1.9us/step)."""
    assert n_t % chunk == 0
    n_chunks = n_t // chunk
    t_all = _time_grid(n_t)
    k_drift = float(np.float32(1.0) + np.float32(R_RATE) * np.float32(DT))

    nc = bacc.Bacc("TRN2", target_bir_lowering=False, debug=False,
                   num_devices=N_CORES)
    # Register a const AP for the Ln bias (activation converts float biases
    # to per-partition const APs; only 0.0/1.0 are pre-registered).
    _const = nc.alloc_sbuf_tensor(f"const-f32-{X_SHIFT}", [P, 1],
                                  mybir.dt.float32)
    nc.gpsimd.memset(_const.ap(), X_SHIFT)
    nc.const_aps.aps[(mybir.dt.float32, X_SHIFT)] = _const.ap()
    nc.all_engine_barrier()

    dW_ext = nc.dram_tensor("dW", [n_t, P, COLS], mybir.dt.float32,
                            kind="ExternalInput")
    S_ext = nc.dram_tensor("S", [n_t, P, COLS], mybir.dt.float32,
                           kind="ExternalOutput")

    from contextlib import ExitStack
    with tile.TileContext(nc) as tc, ExitStack() as stack:
        if reps > 1:
            stack.enter_context(tc.For_i(0, reps, 1))
        with tc.tile_pool(name="dw", bufs=dw_bufs) as dw_pool, \
             tc.tile_pool(name="out", bufs=o_bufs) as o_pool, \
             tc.tile_pool(name="tmp", bufs=tmp_bufs) as tmp_pool, \
             tc.tile_pool(name="ptmp", bufs=2, space="PSUM") as ptmp_pool:

            dw_prev = None
            a_prev = None
            prev = None  # AP of S_{r-1} tile [128, COLS]
            for c in range(n_chunks):
                dw_t = dw_pool.tile([P, chunk, COLS], mybir.dt.float32, tag="dw")
                nc.sync.dma_start(
                    out=dw_t[:],
                    in_=dW_ext[c * chunk:(c + 1) * chunk].rearrange("k p n -> p k n"),
                )
                a_t = None
                if fast:
                    # A = 0.3*dW + (1+r*dt), batched over the chunk: the
                    # drift+base-vol part of the update, off the per-step
                    # critical path. Emitted in 4 slices: a single
                    # chunk-wide op is ~2.2us of uninterruptible DVE time
                    # that stalls the per-step critical DVE ops at every
                    # chunk boundary (~2.4us ACT gap per chunk in the
                    # timeline sim).
                    a_t = dw_pool.tile([P, chunk, COLS], mybir.dt.float32,
                                       tag="a")
                    qk = chunk // 8
                    for q in range(8):
                        ia = nc.vector.tensor_scalar(
                            a_t[:, q * qk:(q + 1) * qk, :].rearrange(
                                "p k n -> p (k n)"),
                            dw_t[:, q * qk:(q + 1) * qk, :].rearrange(
                                "p k n -> p (k n)"),
                            SIGMA_BASE, k_drift, ALU.mult, ALU.add)
                        if prio:
                            # de-prioritize below every per-step op so the
                            # scheduler only fills true DVE idle with these
                            # (low bass_priority = preferred; auto values
                            # would beat the critical-path H/S' ops).
                            ia.ins.bass_priority = 2_000_000 + c * 8 + q
                o_t = o_pool.tile([P, chunk, COLS], mybir.dt.float32, tag="o")

                if c == 0:
                    nc.vector.memset(o_t[:, 0, :], S0)
                    prev = o_t[:, 0, :]
                    krange = range(1, chunk)
                else:
                    krange = range(0, chunk)

                for k in krange:
                    step = c * chunk + k - 1      # time index of this update
                    if k == 0:
                        dw_slice = dw_prev[:, chunk - 1, :]
                        a_slice = a_prev[:, chunk - 1, :] if fast else None
                    else:
                        dw_slice = dw_t[:, k - 1, :]
                        a_slice = a_t[:, k - 1, :] if fast else None
                    c_t = float(np.float32(t_all[step]) + np.float32(T_SHIFT))

                    base = 1_000_000 + (c * chunk + k) * 100
                    if wide:
                        # Fewer, larger instructions: per-half Ln (so each
                        # half's chain closes independently), one wide
                        # Exp(0.5L)=sqrt(u), per-half Exp(-c*r); wide DVE
                        # precompute, per-half 2-op critical tail.
                        Lw = tmp_pool.tile([P, COLS], mybir.dt.float32, tag="Lw")
                        iL0 = nc.scalar.activation(Lw[:, 0:HALF], prev[:, 0:HALF],
                                                   AF.Ln, bias=X_SHIFT,
                                                   scale=1.0 / S0)
                        iL1 = nc.scalar.activation(Lw[:, HALF:COLS],
                                                   prev[:, HALF:COLS],
                                                   AF.Ln, bias=X_SHIFT,
                                                   scale=1.0 / S0)
                        Rw = tmp_pool.tile([P, COLS], mybir.dt.float32, tag="Rw")
                        iR = nc.scalar.activation(Rw[:], Lw[:], AF.Exp,
                                                  bias=0.0, scale=0.5)
                        W2w = tmp_pool.tile([P, COLS], mybir.dt.float32, tag="W2w")
                        jW = nc.vector.tensor_tensor(W2w[:], dw_slice[:], prev,
                                                     ALU.mult)
                        ASw = tmp_pool.tile([P, COLS], mybir.dt.float32, tag="ASw")
                        jA = nc.vector.tensor_tensor(ASw[:], a_slice[:], prev,
                                                     ALU.mult)
                        B2Sw = tmp_pool.tile([P, COLS], mybir.dt.float32, tag="B2Sw")
                        jB = nc.vector.scalar_tensor_tensor(B2Sw[:], Rw[:], c_t,
                                                            W2w[:], ALU.mult,
                                                            ALU.mult)
                        if prio:
                            iL0.ins.bass_priority = base + 0
                            iL1.ins.bass_priority = base + 1
                            iR.ins.bass_priority = base + 2
                            jW.ins.bass_priority = base + 3
                            jA.ins.bass_priority = base + 4
                            jB.ins.bass_priority = base + 5
                        for h in range(2):
                            cs = slice(HALF * h, HALF * (h + 1))
                            E = tmp_pool.tile([P, HALF], mybir.dt.float32,
                                              tag=f"E{h}")
                            iE = nc.scalar.activation(E[:], Rw[:, cs], AF.Exp,
                                                      bias=0.0, scale=-c_t)
                            Hh = tmp_pool.tile([P, HALF], mybir.dt.float32,
                                               tag=f"H{h}")
                            iH = nc.vector.tensor_tensor(Hh[:], B2Sw[:, cs],
                                                         E[:], ALU.mult)
                            iS = nc.vector.tensor_tensor(o_t[:, k, cs], Hh[:],
                                                         ASw[:, cs], ALU.add)
                            if prio:
                                iE.ins.bass_priority = base + 10 + h
                                iH.ins.bass_priority = base + 20 + 2 * h
                                iS.ins.bass_priority = base + 21 + 2 * h
                        prev = o_t[:, k, :]
                        continue
                    e_prev_half = None
                    for h in range(2):
                        cs = slice(HALF * h, HALF * (h + 1))
                        s_prev = prev[:, cs]
                        # L and r in PSUM: ACT's PSUM port is faster
                        # (172 vs 222 init cycles), shortening the L->R->E
                        # chain on the per-step critical cycle.
                        lpool = ptmp_pool if psum else tmp_pool
                        L = lpool.tile([P, HALF], mybir.dt.float32, tag=f"L{h}")
                        # L = ln(S/S0 + XS)
                        i0 = nc.scalar.activation(L[:], s_prev, AF.Ln,
                                                  bias=X_SHIFT, scale=1.0 / S0)
                        Rt = lpool.tile([P, HALF], mybir.dt.float32, tag=f"R{h}")
                        # r = exp(0.5*L) = sqrt(u)
                        i1 = nc.scalar.activation(Rt[:], L[:], AF.Exp,
                                                  bias=0.0, scale=0.5)
                        E = tmp_pool.tile([P, HALF], mybir.dt.float32, tag=f"E{h}")
                        # e = exp(-c_t * r) = exp(-y)
                        i2 = nc.scalar.activation(E[:], Rt[:], AF.Exp,
                                                  bias=0.0, scale=-c_t)
                        if fast:
                            # Critical path from e is only 2 DVE ops:
                            #   S' = (yc*dW*S)*e + (0.3*dW + k_drift)*S
                            # with W2=dW*S, AS=A*S computed at step start and
                            # yc=c*sqrt(u), B2S=yc*W2 right after the R op.
                            W2 = tmp_pool.tile([P, HALF], mybir.dt.float32,
                                               tag=f"W2{h}")
                            j0 = nc.vector.tensor_tensor(W2[:], dw_slice[:, cs],
                                                         s_prev, ALU.mult)
                            AS = tmp_pool.tile([P, HALF], mybir.dt.float32,
                                               tag=f"AS{h}")
                            j1 = nc.vector.tensor_tensor(AS[:], a_slice[:, cs],
                                                         s_prev, ALU.mult)
                            B2S = tmp_pool.tile([P, HALF], mybir.dt.float32,
                                                tag=f"B2S{h}")
                            # yc*W2 = (c*r)*dW*S, folded into one stt
                            j2 = nc.vector.scalar_tensor_tensor(
                                B2S[:], Rt[:], c_t, W2[:], ALU.mult, ALU.mult)
                            Hh = tmp_pool.tile([P, HALF], mybir.dt.float32,
                                               tag=f"H{h}")
                            i4 = nc.vector.tensor_tensor(Hh[:], B2S[:], E[:],
                                                         ALU.mult)
                            i5 = nc.vector.tensor_tensor(o_t[:, k, cs], Hh[:],
                                                         AS[:], ALU.add)
                            if prio:
                                j0.ins.bass_priority = base + 10 * h + 3
                                j1.ins.bass_priority = base + 10 * h + 4
                                j2.ins.bass_priority = base + 10 * h + 5
                        else:
                            Q = tmp_pool.tile([P, HALF], mybir.dt.float32, tag=f"Q{h}")
                            # q = (r*c_t)*e = y*exp(-y)
                            i3 = nc.vector.scalar_tensor_tensor(Q[:], Rt[:], c_t, E[:],
                                                                ALU.mult, ALU.mult)
                            G = tmp_pool.tile([P, HALF], mybir.dt.float32, tag=f"G{h}")
                            # g = (q + SB)*dW = sigma*dW
                            i4 = nc.vector.scalar_tensor_tensor(G[:], Q[:], SIGMA_BASE,
                                                                dw_slice[:, cs],
                                                                ALU.add, ALU.mult)
                            # S' = (g + (1+r*dt))*S
                            i5 = nc.vector.scalar_tensor_tensor(o_t[:, k, cs], G[:],
                                                                k_drift, s_prev,
                                                                ALU.add, ALU.mult)
                            if prio:
                                i3.ins.bass_priority = base + 20 + 10 * h + 0
                        if prio:
                            i0.ins.bass_priority = base + 10 * h + 0
                            i1.ins.bass_priority = base + 10 * h + 1
                            i2.ins.bass_priority = base + 10 * h + 2
                            i4.ins.bass_priority = base + 20 + 10 * h + 1
                            i5.ins.bass_priority = base + 20 + 10 * h + 2
                        if period is not None and fast:
                            # manual schedule floors (scheduling hints only):
                            # bucketed ACT [L0 L1 R0 R1 E0 E1], DVE critical
                            # tail [H0 H1 S0' S1'] at the end of the period.
                            sb = t0 + (c * chunk + k) * period
                            i0.ins.bass_wait_until_ts = sb + 292 * h
                            i1.ins.bass_wait_until_ts = sb + 584 + 292 * h
                            i2.ins.bass_wait_until_ts = sb + 1168 + 292 * h
                            i4.ins.bass_wait_until_ts = sb + 1745 + 194 * h
                            i5.ins.bass_wait_until_ts = sb + 2133 + 194 * h
                        if chain and e_prev_half is not None:
                            # Half-offset software pipeline: half-1's ACT trio
                            # starts only after half-0's E, so DVE(half-0)
                            # overlaps ACT(half-1). Scheduling-only edge
                            # (same engine, in-order at runtime).
                            add_dep_helper(i0.ins, e_prev_half.ins, sync=False,
                                           reason="half-offset pipeline")
                        e_prev_half = i2
                    prev = o_t[:, k, :]

                store = nc.sync if store_eng == "sync" else nc.scalar
                store.dma_start(
                    out=S_ext[c * chunk:(c + 1) * chunk].rearrange("k p n -> p k n"),
                    in_=o_t[:],
                )
                dw_prev = dw_t
                a_prev = a_t
    _compile_with_one_act_table(nc)
    return nc


def _compile_with_one_act_table(nc):
    """nc.compile() with the ACT table-set list restricted to
    natural_log_exp_and_others. The default greedy insertion pass pairs Ln
    with the natural_log set and Exp with exp_and_others, reloading tables
    twice per step (2x255x1283ns = 654us!). All our activations are Ln/Exp,
    which the combined set covers with a single load at kernel entry.
    Indices into act_info.json's act_func_sets are preserved (other entries
    are emptied, not removed)."""
    target = "natural_log_exp_and_others"
    orig = bacc.get_activation_tables

    def patched(arch):
        full = orig(arch)
        assert target in full, sorted(full)
        return {name: (fns if name == target else set())
                for name, fns in full.items()}

    bacc.get_activation_tables = patched
    try:
        nc.compile()
    finally:
        bacc.get_activation_tables = orig


_CACHED = {}


def _get_nc(n_t=N_T, chunk=16, reps=1, prio=True, chain=False, fast=True):
    key = (n_t, chunk, reps, prio, chain, fast)
    if key not in _CACHED:
        _CACHED[key] = build(n_t, chunk, reps, prio, chain, fast)
    return _CACHED[key]


def _shard(dW):
    """Full dW [N_T, M] -> per-core [N_T, 128, 256] slabs."""
    dW = np.ascontiguousarray(np.asarray(dW, dtype=np.float32))
    n_t = dW.shape[0]
    slabs = []
    for c in range(N_CORES):
        slab = dW[:, c * M_CORE:(c + 1) * M_CORE].reshape(n_t, P, COLS)
        slabs.append(np.ascontiguousarray(slab))
    return slabs


def _unshard(results, n_t):
    outs = [np.asarray(r["S"]).reshape(n_t, M_CORE) for r in results]
    return np.concatenate(outs, axis=1)


def run(dW, trace=False, chunk=16):
    """Run the SPMD kernel on 8 cores. Returns (S_full, BassKernelResults)."""
    dW = np.asarray(dW, dtype=np.float32)
    n_t = dW.shape[0]
    nc = _get_nc(n_t, chunk)
    in_maps = [{"dW": slab} for slab in _shard(dW)]
    res = run_bass_kernel_spmd(nc, in_maps, core_ids=list(range(N_CORES)),
                               trace=trace)
    return _unshard(res.results, n_t), res


def kernel(dW):
    out, _ = run(dW, trace=False)
    return out
